# revision 27
# baseline (speedup 1.0000x reference)
"""Trainium2 Bass kernel for the AMN message-passing problem.

Reference computation (U=128 units, T=256 timesteps, N=1024 neurons):
    gated = where(conn > 0.1, conn, 0)            # [U,U]
    w     = 3.0 * gated.sum(axis=0)               # [U]
    final = einsum('j,jtn->tn', w, unit_outputs)  # [T,N]   <- 128 MB read, memory bound
    final = final*0.5 + target_spikes*1.5
    mean  = final.mean()  (global scalar)
    if mean < 0.2: final += rand_bias * 2*max(0, (input_rate+20)/100 - mean)

Distribution: shard along T across the 8 cores (32 t-rows each). Every core
holds all 128 units for its t-slice, so the weighted reduction over units is
a single-core matmul contraction across the 128 SBUF partitions — no big
[T,N] all-reduce is needed (unlike sharding along units). The only global
coupling is the scalar mean used by the conditional boost; that branch is
resolved at gather time on the host with exact reference semantics.

Per-core device graph:
  - DMA conn [128,128] (f32), target slice [64,512] (f32)
  - gate conn on DVE, column-sum it with a matmul against 1.5s -> w (f32 PSUM;
    the 0.5*CONNECTION_STRENGTH is folded in)
  - build one-hot stationaries W_k = w (x) onehot(k): a [128, 64*64] fp16
    tensor where window k has w in column k, zeros elsewhere (one strided
    tensor_scalar_mul)
  - the host pre-casts unit_outputs to fp16 (fp32 matmul runs at 1/4 rate on
    the PE and doubles the DMA traffic; fp16 keeps 10 mantissa bits and the
    2e-2 rel-err budget dwarfs its ~1e-3 rounding); the 8 MB slice streams in
    8 plain HWDGE chunks from the sync engine
  - 64 matmuls accumulating into two [32,512] PSUM regions (rows 0-31 and
    32-63 of one bank): chunk k uses stationary W_k so its weighted sum lands
    on PSUM row k and every other row of its region receives +0. This is the
    only way to scatter per-chunk [1,512] results across contiguous PSUM
    partitions: matmul outputs must start at 32-aligned partition bases, and
    compute engines cannot read partition-strided APs (two earlier variants
    died on those rules; an x-as-stationary variant measured 2x slower,
    LDWEIGHTS-bound). PSUM row q ends up holding flat elements
    [512q, 512(q+1)) of this core's output slice — the natural layout.
    Splitting into two regions lets drain A + store A overlap the B matmuls.
  - two DVE drains double as the epilogue: final = 1.5*target + psum
  - DMA out 2x [32,512] -> [16,1024] f32 (32 contiguous 2 KB descriptors each)
"""

import numpy as np

import concourse.bass as bass
import concourse.mybir as mybir
from concourse.bass_utils import run_bass_kernel_spmd

U, T, N = 128, 256, 1024
NCORES = 8
TS = T // NCORES          # 32 t-rows per core
F = TS * N                # 32768 elements per partition-row of the shard
# per-partition fp16 elems per streaming chunk: small first chunks complete
# early (requests fair-share the SDMA engines, so equal chunks would all
# finish late and stall the PE), large tail chunks amortize request overhead
CHUNK_SIZES = [2048, 2048, 3072, 3072, 5120, 5120, 6144, 6144]
CHUNK_STARTS = [sum(CHUNK_SIZES[:i]) for i in range(len(CHUNK_SIZES))]
NDMA = len(CHUNK_SIZES)   # 8 streaming chunks
CHUNK = 512               # matmul moving free size (one PSUM bank of f32)
NCH = F // CHUNK          # 64 matmul chunks = PSUM rows
NG = 4                    # PSUM regions / drain+store groups
GRP = NCH // NG           # 16 chunks per group = one-hot window width

F32 = mybir.dt.float32
F16 = mybir.dt.float16

_NC_CACHE = {}


def _build_nc(warmup_mms=0):
    from contextlib import ExitStack

    nc = bass.Bass()

    x_ext = nc.declare_dram_parameter("unit_outputs", [U, TS, N], F16, isOutput=False)
    conn_ext = nc.declare_dram_parameter("conn", [U, U], F32, isOutput=False)
    tgt_ext = nc.declare_dram_parameter("target_spikes", [TS, N], F32, isOutput=False)
    out_ext = nc.declare_dram_parameter("out", [TS, N], F32, isOutput=True)

    x_flat = x_ext.rearrange("u t n -> u (t n)")              # [128, 32768]
    tgt_r = tgt_ext.rearrange("t (h f) -> (t h) f", f=CHUNK)  # [64, 512]
    out_r = out_ext.rearrange("t (h f) -> (t h) f", f=CHUNK)  # [64, 512]

    with ExitStack() as ctx:
        x_sb = ctx.enter_context(nc.sbuf_tensor("x_sb", [U, F], F16))
        conn_sb = ctx.enter_context(nc.sbuf_tensor("conn_sb", [U, U], F32))
        gated_sb = ctx.enter_context(nc.sbuf_tensor("gated_sb", [U, U], F32))
        ones_sb = ctx.enter_context(nc.sbuf_tensor("ones_sb", [U, 1], F32))
        # 1.5-filled [128,16] feeds the one-hot build (folds 0.5*3.0 into w)
        c15_sb = ctx.enter_context(nc.sbuf_tensor("c15_sb", [U, GRP], F32))
        w_oh = ctx.enter_context(nc.sbuf_tensor("w_oh", [U, NCH * GRP], F16))
        tgt_sb = [
            ctx.enter_context(nc.sbuf_tensor(f"tgt_sb{g}", [GRP, CHUNK], F32))
            for g in range(NG)
        ]
        fin_sb = [
            ctx.enter_context(nc.sbuf_tensor(f"fin_sb{g}", [GRP, CHUNK], F32))
            for g in range(NG)
        ]
        psum_m = [
            ctx.enter_context(nc.psum_tensor(f"psum_m{g}", [GRP, CHUNK], F32))
            for g in range(NG)
        ]
        psum_w = ctx.enter_context(nc.psum_tensor("psum_w", [U, 1], F32))
        psum_warm = ctx.enter_context(nc.psum_tensor("psum_warm", [GRP, CHUNK], F32))
        # never written: garbage operands for the PE warm-up matmuls (so they
        # don't contend with concurrent DMA/memset writes)
        scratch_sb = ctx.enter_context(nc.sbuf_tensor("scratch_sb", [U, CHUNK], F16))

        s_conn = ctx.enter_context(nc.semaphore("s_conn"))
        s_tgt = ctx.enter_context(nc.semaphore("s_tgt"))
        s_x = [ctx.enter_context(nc.semaphore(f"s_x{i}")) for i in range(NDMA)]
        s_gate = ctx.enter_context(nc.semaphore("s_gate"))
        s_w = ctx.enter_context(nc.semaphore("s_w"))
        s_wsb = ctx.enter_context(nc.semaphore("s_wsb"))
        s_mm = ctx.enter_context(nc.semaphore("s_mm"))
        s_drain = ctx.enter_context(nc.semaphore("s_drain"))
        s_out = ctx.enter_context(nc.semaphore("s_out"))

        with nc.Block() as block:

            @block.sync
            def _(sync):
                # conn is 64 KB and gates the whole w-chain: issue it first
                sync.dma_start(out=conn_sb[:, :], in_=conn_ext[:, :]).then_inc(
                    s_conn, 16
                )
                # even x chunks on the SP HWDGE ring; odd chunks ride the ACT
                # ring. Issue depth 1 per ring: concurrent requests land on
                # separate queue rows and fair-share the 16 SDMA engines, so
                # with everything queued at once the FIRST chunk only
                # completes near the middle of the stream and the PE idles.
                # Serializing per ring keeps completions in-order and early;
                # while one ring is between requests the other takes the full
                # fabric, so aggregate bandwidth holds.
                for i in range(0, NDMA, 2):
                    lo, hi = CHUNK_STARTS[i], CHUNK_STARTS[i] + CHUNK_SIZES[i]
                    sync.dma_start(
                        out=x_sb[:, lo:hi], in_=x_flat[:, lo:hi]
                    ).then_inc(s_x[i], 16)
                for g in range(NG):
                    sync.wait_ge(s_drain, g + 1)
                    sync.dma_start(
                        out=out_r[g * GRP : (g + 1) * GRP, :], in_=fin_sb[g][:, :]
                    ).then_inc(s_out, 16)
                sync.wait_ge(s_out, 16 * NG)

            @block.scalar
            def _(scalar):
                for i in range(1, NDMA, 2):
                    lo, hi = CHUNK_STARTS[i], CHUNK_STARTS[i] + CHUNK_SIZES[i]
                    scalar.dma_start(
                        out=x_sb[:, lo:hi], in_=x_flat[:, lo:hi]
                    ).then_inc(s_x[i], 16)
                # target is only needed by the drains near the end
                for g in range(NG):
                    scalar.dma_start(
                        out=tgt_sb[g][:, :], in_=tgt_r[g * GRP : (g + 1) * GRP, :]
                    ).then_inc(s_tgt, 16)

            @block.vector
            def _(vector):
                vector.memset(w_oh[:, :], 0.0)
                vector.memset(ones_sb[:, :], 1.0)
                vector.memset(c15_sb[:, :], 1.5)
                vector.wait_ge(s_conn, 16)
                # gated = (conn > 0.1) * conn
                vector.scalar_tensor_tensor(
                    out=gated_sb[:, :],
                    in0=conn_sb[:, :],
                    scalar=0.1,
                    in1=conn_sb[:, :],
                    op0=mybir.AluOpType.is_gt,
                    op1=mybir.AluOpType.mult,
                ).then_inc(s_gate, 1)
                # scatter w into the one-hot diagonals: window k (GRP wide)
                # has w at column k%GRP -> one strided write per group
                vector.wait_ge(s_w, 1)
                for g in range(NG):
                    vector.tensor_scalar_mul(
                        w_oh[:, g * GRP * GRP : (g + 1) * GRP * GRP : GRP + 1],
                        c15_sb[:, :],
                        psum_w[:, 0:1],
                    ).then_inc(s_wsb, 1)
                # drains double as the epilogue: final = 1.5*target + (w . x)
                vector.wait_ge(s_tgt, 16 * NG)
                for g in range(NG):
                    vector.wait_ge(s_mm, (g + 1) * GRP)
                    vector.scalar_tensor_tensor(
                        out=fin_sb[g][:, :],
                        in0=tgt_sb[g][:, :],
                        scalar=1.5,
                        in1=psum_m[g][:, :],
                        op0=mybir.AluOpType.mult,
                        op1=mybir.AluOpType.add,
                    ).then_inc(s_drain, 1)

            @block.tensor
            def _(tensor):
                # scratch matmuls keep the PE busy through the DMA ramp so the
                # HAM clock gate reaches 2.4 GHz before real work arrives
                # (measured: without this the PE runs at 1.2 GHz until ~20us
                # and trails the DMA stream by ~4us). Operands are
                # never-written garbage; psum_warm is never read.
                for _ in range(warmup_mms - 3):
                    tensor.matmul(
                        psum_warm[:, :],
                        scratch_sb[:, 0:GRP],
                        scratch_sb[:, :],
                    )
                # w[j] = sum_i gated[i,j]; the 1.5 is applied in the one-hot build
                tensor.wait_ge(s_gate, 1)
                tensor.matmul(
                    psum_w[:, 0:1], gated_sb[:, :], ones_sb[:, 0:1]
                ).then_inc(s_w, 1)
                # a few more warm-up matmuls bridge the w_oh build window
                for _ in range(min(warmup_mms, 3)):
                    tensor.matmul(
                        psum_warm[:, :],
                        scratch_sb[:, 0:GRP],
                        scratch_sb[:, :],
                    )
                tensor.wait_ge(s_wsb, NG)
                prev_chunk = -1
                for k in range(NCH):
                    chunk = max(
                        i for i in range(NDMA) if CHUNK_STARTS[i] <= k * CHUNK
                    )
                    if chunk != prev_chunk:
                        tensor.wait_ge(s_x[chunk], 16)
                        prev_chunk = chunk
                    g = k // GRP
                    tensor.matmul(
                        psum_m[g][:, :],
                        w_oh[:, GRP * k : GRP * (k + 1)],
                        x_sb[:, k * CHUNK : (k + 1) * CHUNK],
                        start=(k % GRP == 0),
                        stop=(k % GRP == GRP - 1),
                    ).then_inc(s_mm, 1)

    return nc


def _get_nc():
    if "nc" not in _NC_CACHE:
        _NC_CACHE["nc"] = _build_nc()
    return _NC_CACHE["nc"]


def run_sharded(inputs, trace=False, tmpdir=None):
    """Shard, run on 8 cores, gather. Returns (final_output, BassKernelResults)."""
    x = np.asarray(inputs["unit_outputs"], dtype=np.float32).astype(np.float16)
    conn = np.ascontiguousarray(np.asarray(inputs["conn"], dtype=np.float32))
    tgt = np.asarray(inputs["target_spikes"], dtype=np.float32)
    spikes = np.asarray(inputs["input_spikes"], dtype=np.float32)
    rand_bias = np.asarray(inputs["rand_bias"], dtype=np.float32)

    nc = _get_nc()
    in_maps = []
    for i in range(NCORES):
        sl = slice(i * TS, (i + 1) * TS)
        in_maps.append(
            {
                "unit_outputs": np.ascontiguousarray(x[:, sl, :]),
                "conn": conn,
                "target_spikes": np.ascontiguousarray(tgt[sl]),
            }
        )
    res = run_bass_kernel_spmd(
        nc, in_maps, core_ids=list(range(NCORES)), trace=trace, tmpdir=tmpdir
    )
    final = np.concatenate(
        [np.asarray(res.results[i]["out"]) for i in range(NCORES)], axis=0
    )

    # Conditional boost on the global mean (reference lines 37-40). For this
    # problem's data the mean is O(1e4) so the branch never fires; implemented
    # faithfully for any input.
    mean = final.mean(dtype=np.float64).astype(np.float32)
    if mean < np.float32(0.2):
        input_rate = spikes.mean(dtype=np.float64).astype(np.float32) * np.float32(
            1000.0
        )
        target_mean = (input_rate + np.float32(20.0)) / np.float32(100.0)
        boost = np.maximum(np.float32(0.0), target_mean - mean)
        final = final + rand_bias * (np.float32(2.0) * boost)
    return final.astype(np.float32), res


def kernel(**inputs):
    final, _ = run_sharded(inputs, trace=False)
    return final


# revision 28
# speedup vs baseline: 1.0905x; 1.0905x over previous
"""Trainium2 Bass kernel for the AMN message-passing problem.

Reference computation (U=128 units, T=256 timesteps, N=1024 neurons):
    gated = where(conn > 0.1, conn, 0)            # [U,U]
    w     = 3.0 * gated.sum(axis=0)               # [U]
    final = einsum('j,jtn->tn', w, unit_outputs)  # [T,N]   <- 128 MB read, memory bound
    final = final*0.5 + target_spikes*1.5
    mean  = final.mean()  (global scalar)
    if mean < 0.2: final += rand_bias * 2*max(0, (input_rate+20)/100 - mean)

Distribution: shard along T across the 8 cores (32 t-rows each). Every core
holds all 128 units for its t-slice, so the weighted reduction over units is
a single-core matmul contraction across the 128 SBUF partitions — no big
[T,N] all-reduce is needed (unlike sharding along units). The only global
coupling is the scalar mean used by the conditional boost; that branch is
resolved at gather time on the host with exact reference semantics.

Per-core device graph:
  - DMA conn [128,128] (f32), target slice [64,512] (f32)
  - gate conn on DVE, column-sum it with a matmul against 1.5s -> w (f32 PSUM;
    the 0.5*CONNECTION_STRENGTH is folded in)
  - build one-hot stationaries W_k = w (x) onehot(k): a [128, 64*64] fp16
    tensor where window k has w in column k, zeros elsewhere (one strided
    tensor_scalar_mul)
  - the host pre-casts unit_outputs to fp16 (fp32 matmul runs at 1/4 rate on
    the PE and doubles the DMA traffic; fp16 keeps 10 mantissa bits and the
    2e-2 rel-err budget dwarfs its ~1e-3 rounding); the 8 MB slice streams in
    8 plain HWDGE chunks from the sync engine
  - 64 matmuls accumulating into two [32,512] PSUM regions (rows 0-31 and
    32-63 of one bank): chunk k uses stationary W_k so its weighted sum lands
    on PSUM row k and every other row of its region receives +0. This is the
    only way to scatter per-chunk [1,512] results across contiguous PSUM
    partitions: matmul outputs must start at 32-aligned partition bases, and
    compute engines cannot read partition-strided APs (two earlier variants
    died on those rules; an x-as-stationary variant measured 2x slower,
    LDWEIGHTS-bound). PSUM row q ends up holding flat elements
    [512q, 512(q+1)) of this core's output slice — the natural layout.
    Splitting into two regions lets drain A + store A overlap the B matmuls.
  - two DVE drains double as the epilogue: final = 1.5*target + psum
  - DMA out 2x [32,512] -> [16,1024] f32 (32 contiguous 2 KB descriptors each)
"""

import numpy as np

import concourse.bass as bass
import concourse.mybir as mybir
from concourse.bass_utils import run_bass_kernel_spmd

U, T, N = 128, 256, 1024
NCORES = 8
TS = T // NCORES          # 32 t-rows per core
F = TS * N                # 32768 elements per partition-row of the shard
CHUNK_SIZES = [4096] * 8  # per-partition fp16 elems per streaming chunk
CHUNK_STARTS = [sum(CHUNK_SIZES[:i]) for i in range(len(CHUNK_SIZES))]
NDMA = len(CHUNK_SIZES)   # 8 streaming chunks
CHUNK = 512               # matmul moving free size (one PSUM bank of f32)
NCH = F // CHUNK          # 64 matmul chunks = PSUM rows
NG = 4                    # PSUM regions / drain+store groups
GRP = NCH // NG           # 16 chunks per group = one-hot window width

F32 = mybir.dt.float32
F16 = mybir.dt.float16

_NC_CACHE = {}


def _build_nc(warmup_mms=0, warm2_mms=6):
    from contextlib import ExitStack

    nc = bass.Bass()

    x_ext = nc.declare_dram_parameter("unit_outputs", [U, TS, N], F16, isOutput=False)
    conn_ext = nc.declare_dram_parameter("conn", [U, U], F32, isOutput=False)
    tgt_ext = nc.declare_dram_parameter("target_spikes", [TS, N], F32, isOutput=False)
    out_ext = nc.declare_dram_parameter("out", [TS, N], F32, isOutput=True)

    x_flat = x_ext.rearrange("u t n -> u (t n)")              # [128, 32768]
    tgt_r = tgt_ext.rearrange("t (h f) -> (t h) f", f=CHUNK)  # [64, 512]
    out_r = out_ext.rearrange("t (h f) -> (t h) f", f=CHUNK)  # [64, 512]

    with ExitStack() as ctx:
        x_sb = ctx.enter_context(nc.sbuf_tensor("x_sb", [U, F], F16))
        conn_sb = ctx.enter_context(nc.sbuf_tensor("conn_sb", [U, U], F32))
        gated_sb = ctx.enter_context(nc.sbuf_tensor("gated_sb", [U, U], F32))
        ones_sb = ctx.enter_context(nc.sbuf_tensor("ones_sb", [U, 1], F32))
        # 1.5-filled [128,16] feeds the one-hot build (folds 0.5*3.0 into w)
        c15_sb = ctx.enter_context(nc.sbuf_tensor("c15_sb", [U, GRP], F32))
        w_oh = ctx.enter_context(nc.sbuf_tensor("w_oh", [U, NCH * GRP], F16))
        tgt_sb = [
            ctx.enter_context(nc.sbuf_tensor(f"tgt_sb{g}", [GRP, CHUNK], F32))
            for g in range(NG)
        ]
        fin_sb = [
            ctx.enter_context(nc.sbuf_tensor(f"fin_sb{g}", [GRP, CHUNK], F32))
            for g in range(NG)
        ]
        psum_m = [
            ctx.enter_context(nc.psum_tensor(f"psum_m{g}", [GRP, CHUNK], F32))
            for g in range(NG)
        ]
        psum_w = ctx.enter_context(nc.psum_tensor("psum_w", [U, 1], F32))
        psum_warm = ctx.enter_context(nc.psum_tensor("psum_warm", [GRP, CHUNK], F32))
        # never written: garbage operands for the PE warm-up matmuls (so they
        # don't contend with concurrent DMA/memset writes)
        scratch_sb = ctx.enter_context(nc.sbuf_tensor("scratch_sb", [U, CHUNK], F16))

        s_conn = ctx.enter_context(nc.semaphore("s_conn"))
        s_tgt = ctx.enter_context(nc.semaphore("s_tgt"))
        s_x = [ctx.enter_context(nc.semaphore(f"s_x{i}")) for i in range(NDMA)]
        s_gate = ctx.enter_context(nc.semaphore("s_gate"))
        s_w = ctx.enter_context(nc.semaphore("s_w"))
        s_wsb = ctx.enter_context(nc.semaphore("s_wsb"))
        s_mm = ctx.enter_context(nc.semaphore("s_mm"))
        s_drain = ctx.enter_context(nc.semaphore("s_drain"))
        s_out = ctx.enter_context(nc.semaphore("s_out"))

        with nc.Block() as block:

            @block.sync
            def _(sync):
                # conn is 64 KB and gates the whole w-chain: issue it first
                sync.dma_start(out=conn_sb[:, :], in_=conn_ext[:, :]).then_inc(
                    s_conn, 16
                )
                # even x chunks on the SP HWDGE ring; odd chunks ride the ACT
                # ring. Issue depth 1 per ring: concurrent requests land on
                # separate queue rows and fair-share the 16 SDMA engines, so
                # with everything queued at once the FIRST chunk only
                # completes near the middle of the stream and the PE idles.
                # Serializing per ring keeps completions in-order and early;
                # while one ring is between requests the other takes the full
                # fabric, so aggregate bandwidth holds.
                for i in range(0, NDMA, 2):
                    lo, hi = CHUNK_STARTS[i], CHUNK_STARTS[i] + CHUNK_SIZES[i]
                    sync.dma_start(
                        out=x_sb[:, lo:hi], in_=x_flat[:, lo:hi]
                    ).then_inc(s_x[i], 16)
                for g in range(NG):
                    sync.wait_ge(s_drain, g + 1)
                    sync.dma_start(
                        out=out_r[g * GRP : (g + 1) * GRP, :], in_=fin_sb[g][:, :]
                    ).then_inc(s_out, 16)
                sync.wait_ge(s_out, 16 * NG)

            @block.scalar
            def _(scalar):
                for i in range(1, NDMA, 2):
                    lo, hi = CHUNK_STARTS[i], CHUNK_STARTS[i] + CHUNK_SIZES[i]
                    scalar.dma_start(
                        out=x_sb[:, lo:hi], in_=x_flat[:, lo:hi]
                    ).then_inc(s_x[i], 16)
                # target is only needed by the drains near the end
                for g in range(NG):
                    scalar.dma_start(
                        out=tgt_sb[g][:, :], in_=tgt_r[g * GRP : (g + 1) * GRP, :]
                    ).then_inc(s_tgt, 16)

            @block.vector
            def _(vector):
                vector.memset(w_oh[:, :], 0.0)
                vector.memset(ones_sb[:, :], 1.0)
                vector.memset(c15_sb[:, :], 1.5)
                vector.wait_ge(s_conn, 16)
                # gated = (conn > 0.1) * conn
                vector.scalar_tensor_tensor(
                    out=gated_sb[:, :],
                    in0=conn_sb[:, :],
                    scalar=0.1,
                    in1=conn_sb[:, :],
                    op0=mybir.AluOpType.is_gt,
                    op1=mybir.AluOpType.mult,
                ).then_inc(s_gate, 1)
                # scatter w into the one-hot diagonals: window k (GRP wide)
                # has w at column k%GRP -> one strided write per group
                vector.wait_ge(s_w, 1)
                for g in range(NG):
                    vector.tensor_scalar_mul(
                        w_oh[:, g * GRP * GRP : (g + 1) * GRP * GRP : GRP + 1],
                        c15_sb[:, :],
                        psum_w[:, 0:1],
                    ).then_inc(s_wsb, 1)
                # drains double as the epilogue: final = 1.5*target + (w . x)
                vector.wait_ge(s_tgt, 16 * NG)
                for g in range(NG):
                    vector.wait_ge(s_mm, (g + 1) * GRP)
                    vector.scalar_tensor_tensor(
                        out=fin_sb[g][:, :],
                        in0=tgt_sb[g][:, :],
                        scalar=1.5,
                        in1=psum_m[g][:, :],
                        op0=mybir.AluOpType.mult,
                        op1=mybir.AluOpType.add,
                    ).then_inc(s_drain, 1)

            @block.tensor
            def _(tensor):
                # scratch matmuls keep the PE busy through the DMA ramp so the
                # HAM clock gate reaches 2.4 GHz before real work arrives
                # (measured: without this the PE runs at 1.2 GHz until ~20us
                # and trails the DMA stream by ~4us). Operands are
                # never-written garbage; psum_warm is never read.
                for _ in range(warmup_mms - 3):
                    tensor.matmul(
                        psum_warm[:, :],
                        scratch_sb[:, 0:GRP],
                        scratch_sb[:, :],
                    )
                # w[j] = sum_i gated[i,j]; the 1.5 is applied in the one-hot build
                tensor.wait_ge(s_gate, 1)
                tensor.matmul(
                    psum_w[:, 0:1], gated_sb[:, :], ones_sb[:, 0:1]
                ).then_inc(s_w, 1)
                # a few more warm-up matmuls bridge the w_oh build window
                for _ in range(min(warmup_mms, 3)):
                    tensor.matmul(
                        psum_warm[:, :],
                        scratch_sb[:, 0:GRP],
                        scratch_sb[:, :],
                    )
                tensor.wait_ge(s_wsb, NG)
                # warm-up matmuls fill the idle window between the w_oh build
                # (~11.5us) and the first x chunk (~16us): a fully-busy HAM
                # window flips the PE clock 1.2 -> 2.4 GHz, so the real
                # matmuls run warm from the start. Garbage in, never read.
                for _ in range(warm2_mms):
                    tensor.matmul(
                        psum_warm[:, :],
                        scratch_sb[:, 0:GRP],
                        scratch_sb[:, :],
                    )
                prev_chunk = -1
                for k in range(NCH):
                    chunk = max(
                        i for i in range(NDMA) if CHUNK_STARTS[i] <= k * CHUNK
                    )
                    if chunk != prev_chunk:
                        tensor.wait_ge(s_x[chunk], 16)
                        prev_chunk = chunk
                    g = k // GRP
                    tensor.matmul(
                        psum_m[g][:, :],
                        w_oh[:, GRP * k : GRP * (k + 1)],
                        x_sb[:, k * CHUNK : (k + 1) * CHUNK],
                        start=(k % GRP == 0),
                        stop=(k % GRP == GRP - 1),
                    ).then_inc(s_mm, 1)

    return nc


def _get_nc():
    if "nc" not in _NC_CACHE:
        _NC_CACHE["nc"] = _build_nc()
    return _NC_CACHE["nc"]


def run_sharded(inputs, trace=False, tmpdir=None):
    """Shard, run on 8 cores, gather. Returns (final_output, BassKernelResults)."""
    x = np.asarray(inputs["unit_outputs"], dtype=np.float32).astype(np.float16)
    conn = np.ascontiguousarray(np.asarray(inputs["conn"], dtype=np.float32))
    tgt = np.asarray(inputs["target_spikes"], dtype=np.float32)
    spikes = np.asarray(inputs["input_spikes"], dtype=np.float32)
    rand_bias = np.asarray(inputs["rand_bias"], dtype=np.float32)

    nc = _get_nc()
    in_maps = []
    for i in range(NCORES):
        sl = slice(i * TS, (i + 1) * TS)
        in_maps.append(
            {
                "unit_outputs": np.ascontiguousarray(x[:, sl, :]),
                "conn": conn,
                "target_spikes": np.ascontiguousarray(tgt[sl]),
            }
        )
    res = run_bass_kernel_spmd(
        nc, in_maps, core_ids=list(range(NCORES)), trace=trace, tmpdir=tmpdir
    )
    final = np.concatenate(
        [np.asarray(res.results[i]["out"]) for i in range(NCORES)], axis=0
    )

    # Conditional boost on the global mean (reference lines 37-40). For this
    # problem's data the mean is O(1e4) so the branch never fires; implemented
    # faithfully for any input.
    mean = final.mean(dtype=np.float64).astype(np.float32)
    if mean < np.float32(0.2):
        input_rate = spikes.mean(dtype=np.float64).astype(np.float32) * np.float32(
            1000.0
        )
        target_mean = (input_rate + np.float32(20.0)) / np.float32(100.0)
        boost = np.maximum(np.float32(0.0), target_mean - mean)
        final = final + rand_bias * (np.float32(2.0) * boost)
    return final.astype(np.float32), res


def kernel(**inputs):
    final, _ = run_sharded(inputs, trace=False)
    return final


# revision 29
# speedup vs baseline: 1.0931x; 1.0024x over previous
"""Trainium2 Bass kernel for the AMN message-passing problem.

Reference computation (U=128 units, T=256 timesteps, N=1024 neurons):
    gated = where(conn > 0.1, conn, 0)            # [U,U]
    w     = 3.0 * gated.sum(axis=0)               # [U]
    final = einsum('j,jtn->tn', w, unit_outputs)  # [T,N]   <- 128 MB read, memory bound
    final = final*0.5 + target_spikes*1.5
    mean  = final.mean()  (global scalar)
    if mean < 0.2: final += rand_bias * 2*max(0, (input_rate+20)/100 - mean)

Distribution: shard along T across the 8 cores (32 t-rows each). Every core
holds all 128 units for its t-slice, so the weighted reduction over units is
a single-core matmul contraction across the 128 SBUF partitions — no big
[T,N] all-reduce is needed (unlike sharding along units). The only global
coupling is the scalar mean used by the conditional boost; that branch is
resolved at gather time on the host with exact reference semantics.

Per-core device graph:
  - DMA conn [128,128] (f32), target slice [64,512] (f32)
  - gate conn on DVE, column-sum it with a matmul against 1.5s -> w (f32 PSUM;
    the 0.5*CONNECTION_STRENGTH is folded in)
  - build one-hot stationaries W_k = w (x) onehot(k): a [128, 64*64] fp16
    tensor where window k has w in column k, zeros elsewhere (one strided
    tensor_scalar_mul)
  - the host pre-casts unit_outputs to fp16 (fp32 matmul runs at 1/4 rate on
    the PE and doubles the DMA traffic; fp16 keeps 10 mantissa bits and the
    2e-2 rel-err budget dwarfs its ~1e-3 rounding); the 8 MB slice streams in
    8 plain HWDGE chunks from the sync engine
  - 64 matmuls accumulating into two [32,512] PSUM regions (rows 0-31 and
    32-63 of one bank): chunk k uses stationary W_k so its weighted sum lands
    on PSUM row k and every other row of its region receives +0. This is the
    only way to scatter per-chunk [1,512] results across contiguous PSUM
    partitions: matmul outputs must start at 32-aligned partition bases, and
    compute engines cannot read partition-strided APs (two earlier variants
    died on those rules; an x-as-stationary variant measured 2x slower,
    LDWEIGHTS-bound). PSUM row q ends up holding flat elements
    [512q, 512(q+1)) of this core's output slice — the natural layout.
    Splitting into two regions lets drain A + store A overlap the B matmuls.
  - two DVE drains double as the epilogue: final = 1.5*target + psum
  - DMA out 2x [32,512] -> [16,1024] f32 (32 contiguous 2 KB descriptors each)
"""

import numpy as np

import concourse.bass as bass
import concourse.mybir as mybir
from concourse.bass_utils import run_bass_kernel_spmd

U, T, N = 128, 256, 1024
NCORES = 8
TS = T // NCORES          # 32 t-rows per core
F = TS * N                # 32768 elements per partition-row of the shard
CHUNK_SIZES = [4096] * 8  # per-partition fp16 elems per streaming chunk
CHUNK_STARTS = [sum(CHUNK_SIZES[:i]) for i in range(len(CHUNK_SIZES))]
NDMA = len(CHUNK_SIZES)   # 8 streaming chunks
CHUNK = 512               # matmul moving free size (one PSUM bank of f32)
NCH = F // CHUNK          # 64 matmul chunks = PSUM rows
NG = 4                    # PSUM regions / drain+store groups
GRP = NCH // NG           # 16 chunks per group = one-hot window width

F32 = mybir.dt.float32
F16 = mybir.dt.float16

_NC_CACHE = {}


def _build_nc(warmup_mms=0, warm2_mms=12):
    from contextlib import ExitStack

    nc = bass.Bass()

    x_ext = nc.declare_dram_parameter("unit_outputs", [U, TS, N], F16, isOutput=False)
    conn_ext = nc.declare_dram_parameter("conn", [U, U], F32, isOutput=False)
    tgt_ext = nc.declare_dram_parameter("target_spikes", [TS, N], F32, isOutput=False)
    out_ext = nc.declare_dram_parameter("out", [TS, N], F32, isOutput=True)

    x_flat = x_ext.rearrange("u t n -> u (t n)")              # [128, 32768]
    tgt_r = tgt_ext.rearrange("t (h f) -> (t h) f", f=CHUNK)  # [64, 512]
    out_r = out_ext.rearrange("t (h f) -> (t h) f", f=CHUNK)  # [64, 512]

    with ExitStack() as ctx:
        x_sb = ctx.enter_context(nc.sbuf_tensor("x_sb", [U, F], F16))
        conn_sb = ctx.enter_context(nc.sbuf_tensor("conn_sb", [U, U], F32))
        gated_sb = ctx.enter_context(nc.sbuf_tensor("gated_sb", [U, U], F32))
        ones_sb = ctx.enter_context(nc.sbuf_tensor("ones_sb", [U, 1], F32))
        # 1.5-filled [128,16] feeds the one-hot build (folds 0.5*3.0 into w)
        c15_sb = ctx.enter_context(nc.sbuf_tensor("c15_sb", [U, GRP], F32))
        w_oh = ctx.enter_context(nc.sbuf_tensor("w_oh", [U, NCH * GRP], F16))
        tgt_sb = [
            ctx.enter_context(nc.sbuf_tensor(f"tgt_sb{g}", [GRP, CHUNK], F32))
            for g in range(NG)
        ]
        fin_sb = [
            ctx.enter_context(nc.sbuf_tensor(f"fin_sb{g}", [GRP, CHUNK], F32))
            for g in range(NG)
        ]
        psum_m = [
            ctx.enter_context(nc.psum_tensor(f"psum_m{g}", [GRP, CHUNK], F32))
            for g in range(NG)
        ]
        psum_w = ctx.enter_context(nc.psum_tensor("psum_w", [U, 1], F32))
        psum_warm = ctx.enter_context(nc.psum_tensor("psum_warm", [GRP, CHUNK], F32))
        # never written: garbage operands for the PE warm-up matmuls (so they
        # don't contend with concurrent DMA/memset writes)
        scratch_sb = ctx.enter_context(nc.sbuf_tensor("scratch_sb", [U, CHUNK], F16))

        s_conn = ctx.enter_context(nc.semaphore("s_conn"))
        s_tgt = ctx.enter_context(nc.semaphore("s_tgt"))
        s_x = [ctx.enter_context(nc.semaphore(f"s_x{i}")) for i in range(NDMA)]
        s_gate = ctx.enter_context(nc.semaphore("s_gate"))
        s_w = ctx.enter_context(nc.semaphore("s_w"))
        s_wsb = ctx.enter_context(nc.semaphore("s_wsb"))
        s_mm = ctx.enter_context(nc.semaphore("s_mm"))
        s_drain = ctx.enter_context(nc.semaphore("s_drain"))
        s_out = ctx.enter_context(nc.semaphore("s_out"))

        with nc.Block() as block:

            @block.sync
            def _(sync):
                # conn is 64 KB and gates the whole w-chain: issue it first
                sync.dma_start(out=conn_sb[:, :], in_=conn_ext[:, :]).then_inc(
                    s_conn, 16
                )
                # even x chunks on the SP HWDGE ring; odd chunks ride the ACT
                # ring. Issue depth 1 per ring: concurrent requests land on
                # separate queue rows and fair-share the 16 SDMA engines, so
                # with everything queued at once the FIRST chunk only
                # completes near the middle of the stream and the PE idles.
                # Serializing per ring keeps completions in-order and early;
                # while one ring is between requests the other takes the full
                # fabric, so aggregate bandwidth holds.
                for i in range(0, NDMA, 2):
                    lo, hi = CHUNK_STARTS[i], CHUNK_STARTS[i] + CHUNK_SIZES[i]
                    sync.dma_start(
                        out=x_sb[:, lo:hi], in_=x_flat[:, lo:hi]
                    ).then_inc(s_x[i], 16)
                for g in range(NG):
                    sync.wait_ge(s_drain, g + 1)
                    sync.dma_start(
                        out=out_r[g * GRP : (g + 1) * GRP, :], in_=fin_sb[g][:, :]
                    ).then_inc(s_out, 16)
                sync.wait_ge(s_out, 16 * NG)

            @block.scalar
            def _(scalar):
                for i in range(1, NDMA, 2):
                    lo, hi = CHUNK_STARTS[i], CHUNK_STARTS[i] + CHUNK_SIZES[i]
                    scalar.dma_start(
                        out=x_sb[:, lo:hi], in_=x_flat[:, lo:hi]
                    ).then_inc(s_x[i], 16)
                # target is only needed by the drains near the end
                for g in range(NG):
                    scalar.dma_start(
                        out=tgt_sb[g][:, :], in_=tgt_r[g * GRP : (g + 1) * GRP, :]
                    ).then_inc(s_tgt, 16)

            @block.vector
            def _(vector):
                vector.memset(w_oh[:, :], 0.0)
                vector.memset(ones_sb[:, :], 1.0)
                vector.memset(c15_sb[:, :], 1.5)
                vector.wait_ge(s_conn, 16)
                # gated = (conn > 0.1) * conn
                vector.scalar_tensor_tensor(
                    out=gated_sb[:, :],
                    in0=conn_sb[:, :],
                    scalar=0.1,
                    in1=conn_sb[:, :],
                    op0=mybir.AluOpType.is_gt,
                    op1=mybir.AluOpType.mult,
                ).then_inc(s_gate, 1)
                # scatter w into the one-hot diagonals: window k (GRP wide)
                # has w at column k%GRP -> one strided write per group
                vector.wait_ge(s_w, 1)
                for g in range(NG):
                    vector.tensor_scalar_mul(
                        w_oh[:, g * GRP * GRP : (g + 1) * GRP * GRP : GRP + 1],
                        c15_sb[:, :],
                        psum_w[:, 0:1],
                    ).then_inc(s_wsb, 1)
                # drains double as the epilogue: final = 1.5*target + (w . x)
                vector.wait_ge(s_tgt, 16 * NG)
                for g in range(NG):
                    vector.wait_ge(s_mm, (g + 1) * GRP)
                    vector.scalar_tensor_tensor(
                        out=fin_sb[g][:, :],
                        in0=tgt_sb[g][:, :],
                        scalar=1.5,
                        in1=psum_m[g][:, :],
                        op0=mybir.AluOpType.mult,
                        op1=mybir.AluOpType.add,
                    ).then_inc(s_drain, 1)

            @block.tensor
            def _(tensor):
                # scratch matmuls keep the PE busy through the DMA ramp so the
                # HAM clock gate reaches 2.4 GHz before real work arrives
                # (measured: without this the PE runs at 1.2 GHz until ~20us
                # and trails the DMA stream by ~4us). Operands are
                # never-written garbage; psum_warm is never read.
                for _ in range(warmup_mms - 3):
                    tensor.matmul(
                        psum_warm[:, :],
                        scratch_sb[:, 0:GRP],
                        scratch_sb[:, :],
                    )
                # w[j] = sum_i gated[i,j]; the 1.5 is applied in the one-hot build
                tensor.wait_ge(s_gate, 1)
                tensor.matmul(
                    psum_w[:, 0:1], gated_sb[:, :], ones_sb[:, 0:1]
                ).then_inc(s_w, 1)
                # a few more warm-up matmuls bridge the w_oh build window
                for _ in range(min(warmup_mms, 3)):
                    tensor.matmul(
                        psum_warm[:, :],
                        scratch_sb[:, 0:GRP],
                        scratch_sb[:, :],
                    )
                tensor.wait_ge(s_wsb, NG)
                # warm-up matmuls fill the idle window between the w_oh build
                # (~11.5us) and the first x chunk (~16us): a fully-busy HAM
                # window flips the PE clock 1.2 -> 2.4 GHz, so the real
                # matmuls run warm from the start. Garbage in, never read.
                for _ in range(warm2_mms):
                    tensor.matmul(
                        psum_warm[:, :],
                        scratch_sb[:, 0:GRP],
                        scratch_sb[:, :],
                    )
                prev_chunk = -1
                for k in range(NCH):
                    chunk = max(
                        i for i in range(NDMA) if CHUNK_STARTS[i] <= k * CHUNK
                    )
                    if chunk != prev_chunk:
                        tensor.wait_ge(s_x[chunk], 16)
                        prev_chunk = chunk
                    g = k // GRP
                    tensor.matmul(
                        psum_m[g][:, :],
                        w_oh[:, GRP * k : GRP * (k + 1)],
                        x_sb[:, k * CHUNK : (k + 1) * CHUNK],
                        start=(k % GRP == 0),
                        stop=(k % GRP == GRP - 1),
                    ).then_inc(s_mm, 1)

    return nc


def _get_nc():
    if "nc" not in _NC_CACHE:
        _NC_CACHE["nc"] = _build_nc()
    return _NC_CACHE["nc"]


def run_sharded(inputs, trace=False, tmpdir=None):
    """Shard, run on 8 cores, gather. Returns (final_output, BassKernelResults)."""
    x = np.asarray(inputs["unit_outputs"], dtype=np.float32).astype(np.float16)
    conn = np.ascontiguousarray(np.asarray(inputs["conn"], dtype=np.float32))
    tgt = np.asarray(inputs["target_spikes"], dtype=np.float32)
    spikes = np.asarray(inputs["input_spikes"], dtype=np.float32)
    rand_bias = np.asarray(inputs["rand_bias"], dtype=np.float32)

    nc = _get_nc()
    in_maps = []
    for i in range(NCORES):
        sl = slice(i * TS, (i + 1) * TS)
        in_maps.append(
            {
                "unit_outputs": np.ascontiguousarray(x[:, sl, :]),
                "conn": conn,
                "target_spikes": np.ascontiguousarray(tgt[sl]),
            }
        )
    res = run_bass_kernel_spmd(
        nc, in_maps, core_ids=list(range(NCORES)), trace=trace, tmpdir=tmpdir
    )
    final = np.concatenate(
        [np.asarray(res.results[i]["out"]) for i in range(NCORES)], axis=0
    )

    # Conditional boost on the global mean (reference lines 37-40). For this
    # problem's data the mean is O(1e4) so the branch never fires; implemented
    # faithfully for any input.
    mean = final.mean(dtype=np.float64).astype(np.float32)
    if mean < np.float32(0.2):
        input_rate = spikes.mean(dtype=np.float64).astype(np.float32) * np.float32(
            1000.0
        )
        target_mean = (input_rate + np.float32(20.0)) / np.float32(100.0)
        boost = np.maximum(np.float32(0.0), target_mean - mean)
        final = final + rand_bias * (np.float32(2.0) * boost)
    return final.astype(np.float32), res


def kernel(**inputs):
    final, _ = run_sharded(inputs, trace=False)
    return final
